# revision 2
# baseline (speedup 1.0000x reference)
"""DelayBuffer Trainium2 kernel.

Input:  embeddings [4, 4096, 1024] f32.
Output: [4, 4096, 6144] f32 — for each delay d in (1,2,4,8,16,32),
        out_d[t] = emb[t-d] if t >= d else emb[t], concatenated on the
        last axis.

Sharding: 8 cores = (batch b in 0..3) x (embed half h in 0..1). Each
core receives a contiguous [4096, 512] shard and produces [6, 4096*512]
(the six delayed copies of its shard). Host reassembles.

Kernel (pure DMA, hybrid SBUF/D2D): the per-core work is 8 MiB in,
48 MiB out. Two per-core bottlenecks were measured on this part:
  - the SBUF<->DMA port sustains ~190 GB/s (so staging everything
    through SBUF floors at ~300 us),
  - the HBM fabric sustains ~630 GB/s per core but only ~2.4 TB/s
    chip-wide with all 8 cores running (so pure DRAM->DRAM copies,
    which read the source once per delay, floor at ~330 us).
Splitting the six delayed copies between the two paths uses both
resources at once: delays 1,2,4,8 (+ the first 1664 rows of delay 16)
are stored from an SBUF copy of the shard (~35 MiB of HBM writes,
source read once), while the rest of delay 16 and all of delay 32 are
direct DRAM->DRAM copies (~25 MiB reads + writes that skip the SBUF
port). DMA instructions are split across the two fast DMA queues
(gpsimd SWDGE + Activation HWDGE; the SP HWDGE queue measured ~30 GB/s
and slows the tail). D2D copies are issued first — they have no
dependencies and keep the DMA rings busy while the SBUF load lands.
Measured steady-state: ~230 us/core vs ~1010 us for the single-queue
all-SBUF baseline.
"""

import numpy as np

import concourse.bass as bass
import concourse.tile as tile
from concourse import mybir
from concourse.bass_utils import run_bass_kernel_spmd

DELAYS = (1, 2, 4, 8, 16, 32)
B, S, D = 4, 4096, 1024
NCORES = 8
C = 512           # channels per core (half of D)
P = 128           # SBUF partitions
RPP = S // P      # rows per partition = 32
FREE = RPP * C    # floats per partition = 16384

SBUF_DELAYS = (1, 2, 4, 8)   # fully via SBUF stores
MIX_DELAY = 16               # rows [MIX_DELAY, MIX_DELAY+MIX_R) via SBUF
MIX_R = 1664                 # multiple of RPP
D2D_NSEG = 4                 # segments per full D2D delay copy

_cached_nc = None


def _split_multi_waits(nc: bass.Bass) -> None:
    # This walrus version can encode only ONE sync-wait per instruction
    # (the TPB header's single EVENTS slot); codegen aborts with "Too many
    # sync wait commands" otherwise. The Tile kernel-tail drain waits on
    # every DMA sem lane, so split: hoist all but the last wait onto
    # fresh single-wait NoOps inserted just before the instruction on the
    # same engine queue.
    from concourse import mybir

    for f in nc.m.functions:
        for bb in f.blocks:
            new_insts = []
            for inst in bb.instructions:
                si = getattr(inst, "sync_info", None)
                if si is not None and si.on_wait and len(si.on_wait) > 1:
                    for w in si.on_wait[:-1]:
                        nop = mybir.InstNoOp(
                            name=nc.get_next_instruction_name(),
                            engine=inst.engine,
                        )
                        nop.sync_info = mybir.SyncInfo(on_wait=[w], on_update=[])
                        new_insts.append(nop)
                    si.on_wait = [si.on_wait[-1]]
                new_insts.append(inst)
            bb.instructions[:] = new_insts


def _build_program(reps: int = 1) -> bass.Bass:
    # reps > 1 repeats the whole kernel serially inside one NEFF (the
    # shared SBUF tile's WAR/WAW deps force rep i+1's load to wait for
    # rep i's stores) — used only for benchmarking, where the marginal
    # time between two rep counts cancels the multi-ms PJRT dispatch
    # overhead of this axon client.
    F32 = mybir.dt.float32
    nc = bass.Bass()
    x = nc.declare_dram_parameter("x", [S, C], F32, isOutput=False)
    y = nc.declare_dram_parameter(
        "y", [len(DELAYS), S * C], F32, isOutput=True
    )
    pool_e, act_e = nc.gpsimd, nc.scalar
    engs = [pool_e, act_e]
    xf = x.rearrange("s c -> (s c)")
    xr = x.rearrange("(p n) c -> p n c", p=P)
    kof = {d: k for k, d in enumerate(DELAYS)}

    # (delay, src_elem_lo, src_elem_hi) copied DRAM->DRAM
    d2d_work = [
        (d, 0, (S - d) * C)
        for d in DELAYS
        if d not in SBUF_DELAYS and d != MIX_DELAY
    ]
    d2d_work.append((MIX_DELAY, MIX_R * C, (S - MIX_DELAY) * C))

    with tile.TileContext(nc) as tc:
        with tc.tile_pool(name="sbuf", bufs=1) as pool:
            xt = pool.tile([P, FREE], F32)
            qi = 0
            for _ in range(reps):
                # D2D copies first: no dependencies, keep the rings busy
                # while the load lands.
                for d, a0, b0 in d2d_work:
                    yk = y[kof[d]]
                    n = b0 - a0
                    nseg = max(1, round(n / ((S * C) // D2D_NSEG)))
                    bounds = [
                        a0 + (n * i // nseg) // 131072 * 131072
                        for i in range(1, nseg)
                    ]
                    bounds = [a0] + bounds + [b0]
                    for i in range(len(bounds) - 1):
                        if bounds[i] < bounds[i + 1]:
                            engs[qi % 2].dma_start(
                                out=yk[d * C + bounds[i] : d * C + bounds[i + 1]],
                                in_=xf[bounds[i] : bounds[i + 1]],
                            )
                            qi += 1
                    # head: identity rows t < d
                    engs[qi % 2].dma_start(out=yk[0 : d * C], in_=xf[0 : d * C])
                    qi += 1
                # Load shard into SBUF: row r -> partition r//RPP, chunk r%RPP
                pool_e.dma_start(out=xt[0:64, :], in_=xr[0:64])
                act_e.dma_start(out=xt[64:128, :], in_=xr[64:128])
                # SBUF-sourced stores
                for d in SBUF_DELAYS:
                    yk = y[kof[d]]
                    # bulk: partitions [0,64) on Pool, [64,127) on Act
                    pool_e.dma_start(
                        out=yk[d * C : (64 * RPP + d) * C], in_=xt[0:64, :]
                    )
                    act_e.dma_start(
                        out=yk[(64 * RPP + d) * C : (127 * RPP + d) * C],
                        in_=xt[64:127, :],
                    )
                    # tail: partition 127 holds rows (P-1)*RPP..S-1; keep
                    # the first RPP-d, they land at rows (P-1)*RPP+d..S-1
                    engs[qi % 2].dma_start(
                        out=yk[((P - 1) * RPP + d) * C : S * C],
                        in_=xt[P - 1 : P, 0 : (RPP - d) * C],
                    )
                    qi += 1
                    # head: identity rows t < d
                    engs[qi % 2].dma_start(
                        out=yk[0 : d * C], in_=xt[0:1, 0 : d * C]
                    )
                    qi += 1
                # SBUF part of the mixed delay: dst rows [d, d+MIX_R)
                d = MIX_DELAY
                yk = y[kof[d]]
                half = MIX_R // RPP // 2
                pool_e.dma_start(
                    out=yk[d * C : (half * RPP + d) * C], in_=xt[0:half, :]
                )
                act_e.dma_start(
                    out=yk[(half * RPP + d) * C : (MIX_R + d) * C],
                    in_=xt[half : MIX_R // RPP, :],
                )
    _split_multi_waits(nc)
    return nc


def kernel(embeddings: np.ndarray) -> np.ndarray:
    global _cached_nc
    embeddings = np.ascontiguousarray(embeddings, dtype=np.float32)
    assert embeddings.shape == (B, S, D)

    if _cached_nc is None:
        _cached_nc = _build_program()
    nc = _cached_nc

    # Shard: core c -> batch c//2, embed half c%2.
    in_maps = []
    for c in range(NCORES):
        b, h = divmod(c, 2)
        in_maps.append(
            {"x": np.ascontiguousarray(embeddings[b, :, h * C : (h + 1) * C])}
        )

    results = run_bass_kernel_spmd(nc, in_maps, list(range(NCORES))).results

    out = np.empty((B, S, len(DELAYS) * D), dtype=np.float32)
    for c in range(NCORES):
        b, h = divmod(c, 2)
        yk = results[c]["y"].reshape(len(DELAYS), S, C)
        for k in range(len(DELAYS)):
            out[b, :, k * D + h * C : k * D + (h + 1) * C] = yk[k]
    return out
